# revision 7
# baseline (speedup 1.0000x reference)
"""Self-contained Trainium2 Bass kernel for nn_AllSetTransformer_g (8 NeuronCores).

Algorithm (per PMA stage the softmax numerator exp(leaky_relu(alpha)) is folded
into a per-row gather table, so the whole sparse attention-pool becomes
gather + one-hot-matmul segment sums):

  V2E stages (0,2): edges partitioned by src chunk -> local gather from the
    node table, per-core partial sums over ALL hyperedge segments,
    ReduceScatter combines partials; epilogue on the owned M chunk.
  E2V stages (1,3): AllGather the (small) hyperedge table; segments = local
    nodes, fully local gather + segment sum + fused epilogue.
  Readout/classifier: per-core batch one-hot matmul partials + AllReduce.
"""
import sys
sys.path.insert(0, '/opt/trn_rl_repo')
import numpy as np

import concourse.bass as bass
import concourse.tile as tile
from concourse import mybir, bacc
from concourse.bass_utils import run_bass_kernel_spmd
from concourse.masks import make_identity

P = 128
F32 = mybir.dt.float32
F32R = mybir.dt.float32r
BF16 = mybir.dt.bfloat16
I16 = mybir.dt.int16
AF = mybir.ActivationFunctionType
ALU = mybir.AluOpType
NEG_SLOPE = 0.2

RS_BF16 = True       # partial-sum buffer + ReduceScatter in bf16
MM_DT = F32R         # dtype for dense matmuls (f32 data, TF32-like multiply)


class Cfg:
    def __init__(self, N, M, B, E, ncores=8, D=256, H=8, DH=32, CH=128, NC=10,
                 gather_chunk=1024):
        assert N % ncores == 0 and M % ncores == 0
        self.N, self.M, self.B, self.E = N, M, B, E
        self.ncores, self.D, self.H, self.DH, self.CH, self.NC = ncores, D, H, DH, CH, NC
        self.NL = N // ncores
        self.ML = M // ncores
        self.NLp = -(-self.NL // P) * P
        self.MLp = -(-self.ML // P) * P
        self.Mpad_g = ncores * self.MLp
        self.n_mtiles = self.Mpad_g // P
        self.n_ntiles = self.NLp // P
        self.gather_chunk = gather_chunk
        self.DW = D + H          # 264
        self.TW = 384            # bf16 table row width (768B, mult of 256B)


# ------------------------------------------------------------- host prep
def wrap_idx_chunk(flat):
    blk = flat.reshape(-1, 16).T.astype(np.int16)
    return np.tile(blk, (8, 1))


def build_edge_phase(gather_idx_all, seg_global_all, owner, n_segtiles, cfg):
    nco = cfg.ncores
    per_core_tile_edges = []
    counts = np.zeros((nco, n_segtiles), np.int64)
    for c in range(nco):
        m = owner == c
        gi = gather_idx_all[m]
        sg = seg_global_all[m]
        order = np.argsort(sg, kind="stable")
        gi, sg = gi[order], sg[order]
        tile_id = sg // P
        idx_split = np.searchsorted(tile_id, np.arange(1, n_segtiles))
        per_core_tile_edges.append((np.split(gi, idx_split), np.split(sg, idx_split)))
        counts[c] = [len(a) for a in per_core_tile_edges[c][0]]

    blocks_per_tile = np.maximum(1, -(-counts // P)).max(axis=0)
    NB = int(blocks_per_tile.sum())

    gidx_blocks = np.zeros((nco, NB, P), np.int64)
    segrel_blocks = np.full((nco, NB, P), -1.0, np.float32)
    tile_of_block = np.zeros(NB, np.int64)
    b0 = 0
    for t in range(n_segtiles):
        nb = int(blocks_per_tile[t])
        tile_of_block[b0:b0 + nb] = t
        for c in range(nco):
            gi = per_core_tile_edges[c][0][t]
            sg = per_core_tile_edges[c][1][t]
            k = len(gi)
            flat_g = np.zeros(nb * P, np.int64)
            flat_s = np.full(nb * P, -1.0, np.float32)
            flat_g[:k] = gi
            flat_s[:k] = (sg - t * P).astype(np.float32)
            gidx_blocks[c, b0:b0 + nb] = flat_g.reshape(nb, P)
            segrel_blocks[c, b0:b0 + nb] = flat_s.reshape(nb, P)
        b0 += nb

    bpc = cfg.gather_chunk // P
    chunks = []
    b = 0
    while b < NB:
        n = min(bpc, NB - b)
        chunks.append((b, n))
        b += n

    idx_wrapped = []
    for c in range(nco):
        parts = [wrap_idx_chunk(gidx_blocks[c, bs:bs + nb].reshape(-1))
                 for bs, nb in chunks]
        idx_wrapped.append(np.ascontiguousarray(np.concatenate(parts, axis=1)))
    segrel_T = [np.ascontiguousarray(segrel_blocks[c].T) for c in range(nco)]

    return dict(blocks_per_tile=blocks_per_tile, NB=NB, chunks=chunks,
                tile_of_block=tile_of_block, idx_wrapped=idx_wrapped,
                segrel_T=segrel_T)


def host_prep(v2e_src, v2e_dst, all_batch, cfg):
    src = np.asarray(v2e_src).astype(np.int64)
    dst = np.asarray(v2e_dst).astype(np.int64)
    ab = np.asarray(all_batch).astype(np.int64)
    owner = src // cfg.NL
    src_local = src % cfg.NL
    gm = (dst // cfg.ML) * cfg.MLp + (dst % cfg.ML)
    v2e = build_edge_phase(src_local, gm, owner, cfg.n_mtiles, cfg)
    e2v = build_edge_phase(gm, src_local, owner, cfg.n_ntiles, cfg)
    batch_T = []
    cnt = np.bincount(ab, minlength=cfg.B).astype(np.float32)
    for c in range(cfg.ncores):
        b = np.full(cfg.NLp, -1.0, np.float32)
        b[:cfg.NL] = ab[c * cfg.NL:(c + 1) * cfg.NL].astype(np.float32)
        batch_T.append(np.ascontiguousarray(b.reshape(cfg.n_ntiles, P).T))
    return dict(v2e=v2e, e2v=e2v, batch_T=batch_T,
                cnt=np.maximum(cnt, 1.0).astype(np.float32))


def fold_weights(Wk, bk, att, cfg):
    Wk3 = np.asarray(Wk, np.float32).reshape(cfg.D, cfg.H, cfg.DH)
    a = np.asarray(att, np.float32)
    Wk_att = np.einsum('chd,hd->ch', Wk3, a).astype(np.float32)
    bk_att = np.einsum('hd,hd->h', np.asarray(bk, np.float32).reshape(cfg.H, cfg.DH),
                       a).astype(np.float32)
    return Wk_att, bk_att


# ------------------------------------------------------------- graph build
def build_graph(cfg, meta):
    """meta: dict with v2e/e2v block structure (shared across cores)."""
    D, H, DW, TW, B, CH, NC = cfg.D, cfg.H, cfg.DW, cfg.TW, cfg.B, cfg.CH, cfg.NC
    RSDT = BF16 if RS_BF16 else F32
    nc = bacc.Bacc(None, num_devices=cfg.ncores)

    # ---------------- dram parameters
    x0 = nc.declare_dram_parameter("x0", [cfg.NLp, D], F32, isOutput=False)
    pv, pe = meta['v2e'], meta['e2v']
    idx_v2e = nc.declare_dram_parameter("idx_v2e", [P, 8 * pv['NB']], I16, isOutput=False)
    sr_v2e = nc.declare_dram_parameter("segrel_v2e", [P, pv['NB']], F32, isOutput=False)
    idx_e2v = nc.declare_dram_parameter("idx_e2v", [P, 8 * pe['NB']], I16, isOutput=False)
    sr_e2v = nc.declare_dram_parameter("segrel_e2v", [P, pe['NB']], F32, isOutput=False)
    batch_T = nc.declare_dram_parameter("batch_T", [P, cfg.n_ntiles], F32, isOutput=False)
    cnt_p = nc.declare_dram_parameter("cnt", [B, 1], F32, isOutput=False)
    Wcat_p, bcat_p, W1_p, b1_p, W2_p, b2_p = [], [], [], [], [], []
    attr_p, g0_p, b0_p, g1_p, b1r_p = [], [], [], [], []
    for s in range(4):
        Wcat_p.append(nc.declare_dram_parameter(f"Wcat{s}", [D, DW], F32, isOutput=False))
        bcat_p.append(nc.declare_dram_parameter(f"bcat{s}", [1, DW], F32, isOutput=False))
        W1_p.append(nc.declare_dram_parameter(f"W1_{s}", [D, D], F32, isOutput=False))
        b1_p.append(nc.declare_dram_parameter(f"b1_{s}", [1, D], F32, isOutput=False))
        W2_p.append(nc.declare_dram_parameter(f"W2_{s}", [D, D], F32, isOutput=False))
        b2_p.append(nc.declare_dram_parameter(f"b2_{s}", [1, D], F32, isOutput=False))
        attr_p.append(nc.declare_dram_parameter(f"attrep{s}", [P, D], F32, isOutput=False))
        g0_p.append(nc.declare_dram_parameter(f"g0rep{s}", [P, D], F32, isOutput=False))
        b0_p.append(nc.declare_dram_parameter(f"b0rep{s}", [P, D], F32, isOutput=False))
        g1_p.append(nc.declare_dram_parameter(f"g1rep{s}", [P, D], F32, isOutput=False))
        b1r_p.append(nc.declare_dram_parameter(f"b1rep{s}", [P, D], F32, isOutput=False))
    cW1_p = nc.declare_dram_parameter("cW1", [D, CH], F32, isOutput=False)
    cb1_p = nc.declare_dram_parameter("cb1", [1, CH], F32, isOutput=False)
    clng_p = nc.declare_dram_parameter("clngrep", [B, CH], F32, isOutput=False)
    clnb_p = nc.declare_dram_parameter("clnbrep", [B, CH], F32, isOutput=False)
    cW2_p = nc.declare_dram_parameter("cW2", [CH, NC], F32, isOutput=False)
    cb2_p = nc.declare_dram_parameter("cb2", [1, NC], F32, isOutput=False)
    out_p = nc.declare_dram_parameter("out", [B, NC], F32, isOutput=True)

    # ---------------- internal dram
    table_n = nc.dram_tensor("table_n", [cfg.NLp, TW], BF16)
    table_e_loc = nc.dram_tensor("table_e_loc", [cfg.MLp, TW], BF16)
    table_e_glob = nc.dram_tensor("table_e_glob", [cfg.Mpad_g, TW], BF16,
                                  addr_space="Shared")
    partial = nc.dram_tensor("partial", [cfg.Mpad_g, DW], RSDT)
    rs_out = nc.dram_tensor("rs_out", [cfg.MLp, DW], RSDT)
    x_e = nc.dram_tensor("x_e", [cfg.MLp, D], F32)
    x_n = nc.dram_tensor("x_n", [cfg.NLp, D], F32)
    r_in = nc.dram_tensor("r_in", [B, D], F32)
    r_out = nc.dram_tensor("r_out", [B, D], F32, addr_space="Shared")
    rgroups = [list(range(cfg.ncores))]

    with tile.TileContext(nc) as tc:
        import contextlib
        with contextlib.ExitStack() as ctx:
            cst = ctx.enter_context(tc.tile_pool(name="cst", bufs=1))
            stc = ctx.enter_context(tc.tile_pool(name="stc", bufs=2))   # stage consts
            sb = ctx.enter_context(tc.tile_pool(name="sb", bufs=3))     # working tiles
            ep = ctx.enter_context(tc.tile_pool(name="ep", bufs=3))     # epilogue tiles
            gp = ctx.enter_context(tc.tile_pool(name="gp", bufs=3))     # gathered tiles
            ip = ctx.enter_context(tc.tile_pool(name="ip", bufs=3))     # idx tiles
            oh = ctx.enter_context(tc.tile_pool(name="oh", bufs=4))     # one-hot tiles
            pst = ctx.enter_context(tc.tile_pool(name="pst", bufs=2, space="PSUM"))
            psm = ctx.enter_context(tc.tile_pool(name="psm", bufs=2, space="PSUM"))
            psb = ctx.enter_context(tc.tile_pool(name="psb", bufs=2, space="PSUM"))

            ident = cst.tile([P, P], F32)
            make_identity(nc, ident[:])
            colidx = cst.tile([P, P], F32)
            iot = cst.tile([P, P], mybir.dt.int32)
            nc.gpsimd.iota(iot[:], pattern=[[1, P]], base=0, channel_multiplier=0)
            nc.vector.tensor_copy(out=colidx[:], in_=iot[:])
            ones1 = cst.tile([1, P], F32)
            nc.vector.memset(ones1[:], 1.0)
            ones_r = cst.tile([1, P], F32R)
            nc.vector.tensor_copy(out=ones_r[:], in_=ones1[:])
            eps_t = cst.tile([P, 1], F32)
            nc.vector.memset(eps_t[:], 1e-5)
            tiny_t = cst.tile([P, 1], F32)
            nc.vector.memset(tiny_t[:], 1e-20)
            racc = cst.tile([B, D], F32)
            nc.vector.memset(racc[:], 0.0)

            def load_stage_consts(s):
                w = {}
                for nm, prm, sh in (("Wcat", Wcat_p[s], [P, 2, DW]),
                                    ("W1", W1_p[s], [P, 2, D]),
                                    ("W2", W2_p[s], [P, 2, D])):
                    tmp = stc.tile(sh, F32, tag=nm + "f", name=nm + "f")
                    nc.sync.dma_start(out=tmp[:, 0, :], in_=prm[0:P, :])
                    nc.sync.dma_start(out=tmp[:, 1, :], in_=prm[P:2 * P, :])
                    t = stc.tile(sh, F32R, tag=nm, name=nm)
                    nc.vector.tensor_copy(out=t[:], in_=tmp[:])
                    w[nm] = t
                for nm, prm, wd in (("bcat", bcat_p[s], DW), ("b1", b1_p[s], D),
                                    ("b2", b2_p[s], D)):
                    tmp = stc.tile([1, wd], F32, tag=nm + "f", name=nm + "f")
                    nc.sync.dma_start(out=tmp[:], in_=prm[:])
                    t = stc.tile([1, wd], F32R, tag=nm, name=nm)
                    nc.vector.tensor_copy(out=t[:], in_=tmp[:])
                    w[nm] = t
                for nm, prm in (("att", attr_p[s]), ("g0", g0_p[s]), ("b0", b0_p[s]),
                                ("g1", g1_p[s]), ("b1r", b1r_p[s])):
                    t = stc.tile([P, D], F32, tag=nm, name=nm)
                    nc.sync.dma_start(out=t[:], in_=prm[:])
                    w[nm] = t
                return w

            def transpose128(src_ap, k):
                """[128,128] f32 slice -> transposed sbuf tile (f32r)."""
                tp = pst.tile([P, P], F32, tag="tp")
                nc.tensor.transpose(out=tp[:], in_=src_ap, identity=ident[:])
                dst = sb.tile([P, P], F32R, tag="tT")
                nc.scalar.copy(out=dst[:], in_=tp[:])
                return dst

            def mm_256(lhs0, lhs1, w, brow):
                """(x^T chunks) @ W + b -> psum [128, W.free]"""
                nfree = w.shape[-1]
                mm = psm.tile([P, nfree], F32, tag="mm", name="mm", padded_shape=[P, DW])
                nc.tensor.matmul(out=mm[:], lhsT=lhs0[:], rhs=w[:, 0, :],
                                 start=True, stop=False)
                nc.tensor.matmul(out=mm[:], lhsT=lhs1[:], rhs=w[:, 1, :],
                                 start=False, stop=False)
                nc.tensor.matmul(out=mm[:], lhsT=ones_r[:], rhs=brow[:],
                                 start=False, stop=True)
                return mm

            def build_table(x_dram, rows_pad, w, table_dram):
                for t in range(rows_pad // P):
                    xt = sb.tile([P, D], F32, tag="xt")
                    nc.sync.dma_start(out=xt[:], in_=x_dram[t * P:(t + 1) * P, :])
                    l0 = transpose128(xt[:, 0:P], 0)
                    l1 = transpose128(xt[:, P:2 * P], 1)
                    mm = mm_256(l0, l1, w['Wcat'], w['bcat'])
                    lr1 = sb.tile([P, H], F32, tag="lr1")
                    nc.scalar.mul(out=lr1[:], in_=mm[:, D:DW], mul=NEG_SLOPE)
                    lr = sb.tile([P, H], F32, tag="lr")
                    nc.vector.tensor_tensor(out=lr[:], in0=mm[:, D:DW], in1=lr1[:],
                                            op=ALU.max)
                    pex = sb.tile([P, H], F32, tag="pex")
                    nc.scalar.activation(out=pex[:], in_=lr[:], func=AF.Exp)
                    tab = sb.tile([P, TW], BF16, tag="tab")
                    for h in range(H):
                        nc.vector.tensor_scalar_mul(
                            out=tab[:, h * 32:(h + 1) * 32],
                            in0=mm[:, h * 32:(h + 1) * 32], scalar1=pex[:, h:h + 1])
                    nc.vector.tensor_copy(out=tab[:, D:DW], in_=pex[:])
                    nc.sync.dma_start(out=table_dram[t * P:(t + 1) * P, :], in_=tab[:])

            def epilogue(sums_ap, w):
                """sums_ap [128, 264] (psum f32 or sbuf bf16) -> x tile [128,256] f32"""
                dmax = ep.tile([P, H], F32, tag="dmax")
                nc.vector.tensor_scalar_max(out=dmax[:], in0=sums_ap[:, D:DW],
                                            scalar1=tiny_t[:])
                rec = ep.tile([P, H], F32, tag="rec")
                nc.vector.reciprocal(out=rec[:], in_=dmax[:])
                t0 = ep.tile([P, D], F32, tag="t0")
                for h in range(H):
                    nc.vector.tensor_scalar_mul(
                        out=t0[:, h * 32:(h + 1) * 32],
                        in0=sums_ap[:, h * 32:(h + 1) * 32], scalar1=rec[:, h:h + 1])
                nc.vector.tensor_add(out=t0[:], in0=t0[:], in1=w['att'][:])

                def ln(x_t, g_t, b_t):
                    st = ep.tile([P, 6], F32, tag="st")
                    nc.vector.bn_stats(out=st[:], in_=x_t[:])
                    mv = ep.tile([P, 2], F32, tag="mv")
                    nc.vector.bn_aggr(out=mv[:], in_=st[:])
                    rstd = ep.tile([P, 1], F32, tag="rstd")
                    nc.scalar.activation(out=rstd[:], in_=mv[:, 1:2], func=AF.Sqrt,
                                         bias=eps_t[:], scale=1.0)
                    nc.vector.reciprocal(out=rstd[:], in_=rstd[:])
                    nc.vector.tensor_scalar(out=x_t[:], in0=x_t[:],
                                            scalar1=mv[:, 0:1], scalar2=rstd[:],
                                            op0=ALU.subtract, op1=ALU.mult)
                    nc.vector.tensor_mul(out=x_t[:], in0=x_t[:], in1=g_t[:])
                    nc.vector.tensor_add(out=x_t[:], in0=x_t[:], in1=b_t[:])

                ln(t0, w['g0'], w['b0'])
                l0 = transpose128(t0[:, 0:P], 0)
                l1 = transpose128(t0[:, P:2 * P], 1)
                mm1 = mm_256(l0, l1, w['W1'], w['b1'])
                r1 = ep.tile([P, D], F32, tag="r1")
                nc.scalar.activation(out=r1[:], in_=mm1[:], func=AF.Relu)
                l0 = transpose128(r1[:, 0:P], 0)
                l1 = transpose128(r1[:, P:2 * P], 1)
                mm2 = mm_256(l0, l1, w['W2'], w['b2'])
                u = ep.tile([P, D], F32, tag="u")
                nc.scalar.activation(out=u[:], in_=mm2[:], func=AF.Relu)
                nc.vector.tensor_add(out=u[:], in0=u[:], in1=t0[:])
                ln(u, w['g1'], w['b1r'])
                xo = ep.tile([P, D], F32, tag="xo")
                nc.scalar.activation(out=xo[:], in_=u[:], func=AF.Relu)
                return xo

            def edge_phase(ph, idx_param, sr_param, table_dram, consume):
                """consume(t, psum_tile) called once per seg tile."""
                NB = ph['NB']
                tob = ph['tile_of_block']
                srt = cst.tile([P, NB], F32, tag=f"srt{id(ph)%97}", name="srt")
                nc.sync.dma_start(out=srt[:], in_=sr_param[:])
                cur = {}
                col0 = 0
                for bs, nb in ph['chunks']:
                    idxt = ip.tile([P, nb * 8], I16, tag="idxt")
                    nc.gpsimd.dma_start(out=idxt[:], in_=idx_param[:, col0:col0 + nb * 8])
                    col0 += nb * 8
                    g = gp.tile([P, nb, TW], BF16, tag="g")
                    nc.gpsimd.dma_gather(out_ap=g[:], in_ap=table_dram[:],
                                         idxs_ap=idxt[:], num_idxs=nb * P,
                                         num_idxs_reg=nb * P, elem_size=TW)
                    for bl in range(nb):
                        b = bs + bl
                        t = int(tob[b])
                        first = (b == 0) or (int(tob[b - 1]) != t)
                        last = (b == NB - 1) or (int(tob[b + 1]) != t)
                        if first:
                            cur['ps'] = psb.tile([P, DW], F32, tag="segsum", name="segsum", bufs=3)
                        o = oh.tile([P, P], BF16, tag="oh")
                        nc.vector.tensor_tensor(
                            out=o[:], in0=srt[:, b:b + 1].to_broadcast([P, P]),
                            in1=colidx[:], op=ALU.is_equal)
                        nc.tensor.matmul(out=cur['ps'][:], lhsT=o[:],
                                         rhs=g[:, bl, 0:DW],
                                         start=first, stop=last)
                        if last:
                            consume(t, cur['ps'])

            # ======================== stages
            for layer in range(2):
                sv = 2 * layer       # v2e stage index
                se = sv + 1          # e2v stage index

                # ---- V2E ----
                w = load_stage_consts(sv)
                xin = x0 if sv == 0 else x_n
                build_table(xin, cfg.NLp, w, table_n)

                def v2e_consume(t, ps):
                    o = sb.tile([P, DW], RSDT, tag="pout")
                    nc.scalar.copy(out=o[:], in_=ps[:])
                    nc.sync.dma_start(out=partial[t * P:(t + 1) * P, :], in_=o[:])

                edge_phase(meta['v2e'], idx_v2e, sr_v2e, table_n, v2e_consume)
                nc.gpsimd.collective_compute(
                    "ReduceScatter", ALU.add, replica_groups=rgroups,
                    ins=[partial[:].opt()], outs=[rs_out[:].opt()])
                for t in range(cfg.MLp // P):
                    st_in = sb.tile([P, DW], RSDT, tag="rsin")
                    nc.sync.dma_start(out=st_in[:], in_=rs_out[t * P:(t + 1) * P, :])
                    xo = epilogue(st_in[:], w)
                    nc.sync.dma_start(out=x_e[t * P:(t + 1) * P, :], in_=xo[:])

                # ---- E2V ----
                w2 = load_stage_consts(se)
                build_table(x_e, cfg.MLp, w2, table_e_loc)
                nc.gpsimd.collective_compute(
                    "AllGather", ALU.bypass, replica_groups=rgroups,
                    ins=[table_e_loc[:].opt()], outs=[table_e_glob[:].opt()])
                last_stage = (layer == 1)

                def e2v_consume(t, ps, w2=w2, last_stage=last_stage):
                    xo = epilogue(ps[:], w2)
                    if not last_stage:
                        nc.sync.dma_start(out=x_n[t * P:(t + 1) * P, :], in_=xo[:])
                    else:
                        bto = oh.tile([P, B], BF16, tag="bto")
                        nc.vector.tensor_tensor(
                            out=bto[:], in0=bT_tile[:, t:t + 1].to_broadcast([P, B]),
                            in1=colidx[:, 0:B], op=ALU.is_equal)
                        xob = ep.tile([P, D], BF16, tag="xob")
                        nc.vector.tensor_copy(out=xob[:], in_=xo[:])
                        rps = psb.tile([B, D], F32, tag="segsum", name="rps", bufs=3)
                        nc.tensor.matmul(out=rps[:], lhsT=bto[:], rhs=xob[:],
                                         start=True, stop=True)
                        nc.vector.tensor_add(out=racc[:], in0=racc[:], in1=rps[:])

                if last_stage:
                    bT_tile = cst.tile([P, cfg.n_ntiles], F32)
                    nc.sync.dma_start(out=bT_tile[:], in_=batch_T[:])
                edge_phase(meta['e2v'], idx_e2v, sr_e2v, table_e_glob, e2v_consume)

            # ======================== readout + classifier
            nc.sync.dma_start(out=r_in[:], in_=racc[:])
            nc.gpsimd.collective_compute(
                "AllReduce", ALU.add, replica_groups=rgroups,
                ins=[r_in[:].opt()], outs=[r_out[:].opt()])
            rt = sb.tile([B, D], F32)
            nc.sync.dma_start(out=rt[:], in_=r_out[:])
            cntt = sb.tile([B, 1], F32)
            nc.sync.dma_start(out=cntt[:], in_=cnt_p[:])
            rc = sb.tile([B, 1], F32)
            nc.vector.reciprocal(out=rc[:], in_=cntt[:])
            nc.vector.tensor_scalar_mul(out=rt[:], in0=rt[:], scalar1=rc[:])

            cW1t = sb.tile([P, 2, CH], F32)
            nc.sync.dma_start(out=cW1t[:, 0, :], in_=cW1_p[0:P, :])
            nc.sync.dma_start(out=cW1t[:, 1, :], in_=cW1_p[P:2 * P, :])
            cb1t = sb.tile([1, CH], F32)
            nc.sync.dma_start(out=cb1t[:], in_=cb1_p[:])
            cW2t = sb.tile([P, NC], F32)
            nc.sync.dma_start(out=cW2t[:], in_=cW2_p[:])
            cb2t = sb.tile([1, NC], F32)
            nc.sync.dma_start(out=cb2t[:], in_=cb2_p[:])
            clngt = sb.tile([B, CH], F32)
            nc.sync.dma_start(out=clngt[:], in_=clng_p[:])
            clnbt = sb.tile([B, CH], F32)
            nc.sync.dma_start(out=clnbt[:], in_=clnb_p[:])

            # readout^T chunks [128, B]
            rT = sb.tile([P, 2, B], F32)
            for k in range(2):
                tp = pst.tile([P, P], F32, tag="tp")
                nc.tensor.transpose(out=tp[:, 0:B], in_=rt[:, k * P:(k + 1) * P],
                                    identity=ident[0:B, 0:B])
                nc.scalar.copy(out=rT[:, k, :], in_=tp[:, 0:B])
            hmm = psm.tile([B, CH], F32, tag="mm", name="hmm")
            nc.tensor.matmul(out=hmm[:], lhsT=rT[:, 0, :],
                             rhs=cW1t[:, 0, :], start=True, stop=False)
            nc.tensor.matmul(out=hmm[:], lhsT=rT[:, 1, :],
                             rhs=cW1t[:, 1, :], start=False, stop=False)
            nc.tensor.matmul(out=hmm[:], lhsT=ones1[:, 0:B],
                             rhs=cb1t[:], start=False, stop=True)
            hh = sb.tile([B, CH], F32)
            nc.scalar.copy(out=hh[:], in_=hmm[:])
            # LN over CH
            st = sb.tile([B, 6], F32)
            nc.vector.bn_stats(out=st[:], in_=hh[:])
            mv = sb.tile([B, 2], F32)
            nc.vector.bn_aggr(out=mv[:], in_=st[:])
            rstd = sb.tile([B, 1], F32)
            nc.scalar.activation(out=rstd[:], in_=mv[:, 1:2], func=AF.Sqrt,
                                 bias=eps_t[0:B, :], scale=1.0)
            nc.vector.reciprocal(out=rstd[:], in_=rstd[:])
            nc.vector.tensor_scalar(out=hh[:], in0=hh[:], scalar1=mv[:, 0:1],
                                    scalar2=rstd[:], op0=ALU.subtract, op1=ALU.mult)
            nc.vector.tensor_mul(out=hh[:], in0=hh[:], in1=clngt[:])
            nc.vector.tensor_add(out=hh[:], in0=hh[:], in1=clnbt[:])
            nc.scalar.activation(out=hh[:], in_=hh[:], func=AF.Relu)
            hT = sb.tile([P, B], F32)
            tp = pst.tile([P, P], F32, tag="tp")
            nc.tensor.transpose(out=tp[:, 0:B], in_=hh[:], identity=ident[0:B, 0:B])
            nc.scalar.copy(out=hT[:], in_=tp[:, 0:B])
            omm = psm.tile([B, NC], F32, tag="mm", name="omm")
            nc.tensor.matmul(out=omm[:], lhsT=hT[:],
                             rhs=cW2t[:], start=True, stop=False)
            nc.tensor.matmul(out=omm[:], lhsT=ones1[:, 0:B],
                             rhs=cb2t[:], start=False, stop=True)
            ot = sb.tile([B, NC], F32)
            nc.scalar.copy(out=ot[:], in_=omm[:])
            nc.sync.dma_start(out=out_p[:], in_=ot[:])

    nc.finalize()
    return nc


# ------------------------------------------------------------- runner
def build_in_maps(inputs, prep, cfg):
    X = np.asarray(inputs['X'], np.float32)
    maps = []
    shared = {}
    for s in range(4):
        Wk_att, bk_att = fold_weights(inputs['Wk'][s], inputs['bk'][s],
                                      inputs['att'][s], cfg)
        Wcat = np.concatenate([np.asarray(inputs['Wv'][s], np.float32), Wk_att],
                              axis=1).astype(np.float32)
        bcat = np.concatenate([np.asarray(inputs['bv'][s], np.float32),
                               bk_att]).astype(np.float32)[None, :]
        shared[f"Wcat{s}"] = np.ascontiguousarray(Wcat)
        shared[f"bcat{s}"] = np.ascontiguousarray(bcat)
        shared[f"W1_{s}"] = np.ascontiguousarray(np.asarray(inputs['W1'][s], np.float32))
        shared[f"b1_{s}"] = np.asarray(inputs['b1'][s], np.float32)[None, :]
        shared[f"W2_{s}"] = np.ascontiguousarray(np.asarray(inputs['W2'][s], np.float32))
        shared[f"b2_{s}"] = np.asarray(inputs['b2'][s], np.float32)[None, :]
        shared[f"attrep{s}"] = np.tile(np.asarray(inputs['att'][s], np.float32)
                                       .reshape(1, cfg.D), (P, 1))
        shared[f"g0rep{s}"] = np.tile(np.asarray(inputs['ln0g'][s], np.float32)[None, :], (P, 1))
        shared[f"b0rep{s}"] = np.tile(np.asarray(inputs['ln0b'][s], np.float32)[None, :], (P, 1))
        shared[f"g1rep{s}"] = np.tile(np.asarray(inputs['ln1g'][s], np.float32)[None, :], (P, 1))
        shared[f"b1rep{s}"] = np.tile(np.asarray(inputs['ln1b'][s], np.float32)[None, :], (P, 1))
    shared["cW1"] = np.ascontiguousarray(np.asarray(inputs['cW1'], np.float32))
    shared["cb1"] = np.asarray(inputs['cb1'], np.float32)[None, :]
    shared["cW2"] = np.ascontiguousarray(np.asarray(inputs['cW2'], np.float32))
    shared["cb2"] = np.asarray(inputs['cb2'], np.float32)[None, :]
    shared["clngrep"] = np.tile(np.asarray(inputs['clng'], np.float32)[None, :], (cfg.B, 1))
    shared["clnbrep"] = np.tile(np.asarray(inputs['clnb'], np.float32)[None, :], (cfg.B, 1))
    shared["cnt"] = prep['cnt'][:, None]

    for c in range(cfg.ncores):
        x0 = np.zeros((cfg.NLp, cfg.D), np.float32)
        x0[:cfg.NL] = X[c * cfg.NL:(c + 1) * cfg.NL]
        m = dict(shared)
        m["x0"] = x0
        m["idx_v2e"] = prep['v2e']['idx_wrapped'][c]
        m["segrel_v2e"] = prep['v2e']['segrel_T'][c]
        m["idx_e2v"] = prep['e2v']['idx_wrapped'][c]
        m["segrel_e2v"] = prep['e2v']['segrel_T'][c]
        m["batch_T"] = prep['batch_T'][c]
        maps.append(m)
    return maps


def run_kernel(inputs, cfg, trace=False):
    prep = host_prep(inputs['v2e_src'], inputs['v2e_dst'], inputs['all_batch'], cfg)
    meta = dict(v2e=prep['v2e'], e2v=prep['e2v'])
    nc = build_graph(cfg, meta)
    in_maps = build_in_maps(inputs, prep, cfg)
    res = run_bass_kernel_spmd(nc, in_maps, list(range(cfg.ncores)), trace=trace)
    out = np.asarray(res.results[0]["out"], np.float32)
    return out, res


def kernel(**inputs):
    cfg = Cfg(N=100000, M=20000, B=64, E=400000)
    out, _ = run_kernel(inputs, cfg)
    return out
